# revision 14
# baseline (speedup 1.0000x reference)
"""Trainium2 Bass kernel for nn_AutonomousSystem_89644557402406.

Computation (see reference):
    S    = su2_operators(q_phases)                 # [64, 2, 2] complex
    eta  = mean(angle(det S)) / pi                 # scalar (det==1 analytically)
    x_out = per-quaternion complex 2x2 rotation of x
          = x @ R^T  with R = blockdiag of 64 real 4x4 blocks
    out  = tanh(x_out @ proj_w.T + proj_b) + x_out

Key restructuring: fold the rotation into the projection on the host:
    W_fused = proj_w @ R   =>   out = tanh(x @ W_fused^T + b) + x @ R^T
so the device does a single [256 -> 512] matmul per row block
(columns 0:256 = pre-tanh z, columns 256:512 = x_out), a tanh and an add.

Sharding: pure data parallel over batch across 8 cores (65536 rows/core).
"""

import math

import numpy as np

MANIFOLD_DIM = 256
NQ = 64
BITS = 4
LEVELS = 2 ** BITS
SCALE = 2.0 * math.pi / (LEVELS - 1)
BATCH = 524288

N_CORES = 8
ROWS_PER_CORE = BATCH // N_CORES          # 65536
P = 128                                    # partitions
SUBTILES = 8                               # 128-row subtiles per outer tile
OUTER_ROWS = P * SUBTILES                  # 1024 rows per outer tile
N_OUTER = ROWS_PER_CORE // OUTER_ROWS      # 64


# ---------------------------------------------------------------------------
# Host-side math: rotation matrix R, fused weights, eta
# ---------------------------------------------------------------------------

def _su2_numpy(q_phases, dtype=np.float64):
    ph = q_phases.astype(dtype) * SCALE
    alpha, beta, gamma = ph[:, 0], ph[:, 1], ph[:, 2]
    cb = np.cos(beta / 2)
    sb = np.sin(beta / 2)
    u00 = np.exp(1j * (alpha + gamma) / 2) * cb
    u01 = np.exp(1j * (alpha - gamma) / 2) * sb
    u10 = -np.exp(-1j * (alpha - gamma) / 2) * sb
    u11 = np.exp(-1j * (alpha + gamma) / 2) * cb
    return np.stack(
        [np.stack([u00, u01], -1), np.stack([u10, u11], -1)], -2
    )  # [NQ, 2, 2]


def _rotation_matrix(q_phases):
    """R such that x_out = x @ R.T (R is block-diagonal, 64 blocks of 4x4)."""
    S = _su2_numpy(q_phases)
    Sr, Si = S.real, S.imag
    blk = np.zeros((NQ, 4, 4), dtype=np.float64)
    # in-block ordering: index 2*j + d with j = complex component, d = re/im
    blk[:, 0::2, 0::2] = Sr
    blk[:, 0::2, 1::2] = -Si
    blk[:, 1::2, 0::2] = Si
    blk[:, 1::2, 1::2] = Sr
    R = np.zeros((MANIFOLD_DIM, MANIFOLD_DIM), dtype=np.float64)
    for q in range(NQ):
        R[4 * q : 4 * q + 4, 4 * q : 4 * q + 4] = blk[q]
    return R


def _compute_eta(q_phases):
    """Replicate the reference eta computation as closely as possible."""
    try:
        import jax
        import jax.numpy as jnp

        cpu = jax.local_devices(backend="cpu")[0]
        with jax.default_device(cpu):
            phases = jnp.asarray(q_phases).astype(jnp.float32) * SCALE
            alpha, beta, gamma = phases[:, 0], phases[:, 1], phases[:, 2]
            cb = jnp.cos(beta / 2)
            sb = jnp.sin(beta / 2)
            u00 = jnp.exp(1j * (alpha + gamma) / 2) * cb
            u01 = jnp.exp(1j * (alpha - gamma) / 2) * sb
            u10 = -jnp.exp(-1j * (alpha - gamma) / 2) * sb
            u11 = jnp.exp(-1j * (alpha + gamma) / 2) * cb
            det = u00 * u11 - u01 * u10
            eta = jnp.angle(det).mean() / math.pi
            return np.asarray(eta, dtype=np.float32)
    except Exception:
        S = _su2_numpy(q_phases, dtype=np.float32).astype(np.complex64)
        det = S[:, 0, 0] * S[:, 1, 1] - S[:, 0, 1] * S[:, 1, 0]
        return np.float32(np.angle(det).mean() / np.float32(math.pi))


# ---------------------------------------------------------------------------
# Bass kernel (per-core program; same NEFF on all 8 cores)
# ---------------------------------------------------------------------------

_CACHED_NC = None


def _build_nc():
    global _CACHED_NC
    if _CACHED_NC is not None:
        return _CACHED_NC

    import concourse.mybir as mybir
    from concourse import bacc
    from concourse.tile import TileContext

    f32 = mybir.dt.float32
    bf16 = mybir.dt.bfloat16

    nc = bacc.Bacc("TRN2", target_bir_lowering=False, debug=False)

    # x is f32 in HBM; the input DMA casts to bf16 (SWDGE cast-on-copy).
    # Matmul inputs are bf16 (1 cycle/row on the PE); accumulation is f32.
    x = nc.dram_tensor("x", [ROWS_PER_CORE, MANIFOLD_DIM], f32, kind="ExternalInput")
    # pre-arranged combined weights: wc[p, c, n] = Wc[c*128 + p, n]
    wc = nc.dram_tensor("wc", [P, 2, 512], bf16, kind="ExternalInput")
    bias = nc.dram_tensor("bias", [1, 256], bf16, kind="ExternalInput")
    out = nc.dram_tensor("out", [ROWS_PER_CORE, MANIFOLD_DIM], f32, kind="ExternalOutput")

    with TileContext(nc) as tc:
        with (
            tc.tile_pool(name="const", bufs=1) as cpool,
            tc.tile_pool(name="xin", bufs=3) as xin_pool,
            tc.tile_pool(name="xt", bufs=6) as xt_pool,
            tc.tile_pool(name="th", bufs=6) as th_pool,
            tc.tile_pool(name="res", bufs=3) as res_pool,
            tc.tile_pool(name="psm_p", bufs=6, space="PSUM") as psm_pool,
        ):
            wc_t = cpool.tile([P, 2, 512], bf16)
            nc.sync.dma_start(wc_t[:], wc[:, :, :])
            bias_t = cpool.tile([1, 256], bf16)
            nc.sync.dma_start(bias_t[:], bias[:, :])
            ones_t = cpool.tile([1, P], bf16)
            nc.gpsimd.memset(ones_t[:], 1.0)

            for o in range(N_OUTER):
                row0 = o * OUTER_ROWS
                x_view = x[row0 : row0 + OUTER_ROWS, :].rearrange(
                    "(t p) f -> p t f", p=P
                )
                out_view = out[row0 : row0 + OUTER_ROWS, :].rearrange(
                    "(t p) f -> p t f", p=P
                )

                x_t = xin_pool.tile([P, SUBTILES, MANIFOLD_DIM], bf16)
                # SWDGE dma casts f32 -> bf16 on the fly
                nc.gpsimd.dma_start(out=x_t[:], in_=x_view)
                res = res_pool.tile([P, SUBTILES, MANIFOLD_DIM], f32)

                for t in range(SUBTILES):
                    # transpose the 128x256 subtile -> [feat, batch] via the
                    # DMA xbar (SBUF -> SBUF, bf16)
                    xt_s = xt_pool.tile([P, 256], bf16)
                    nc.sync.dma_start(
                        xt_s[:, 0:128], x_t[:, t, 0:128], transpose=True
                    )
                    nc.sync.dma_start(
                        xt_s[:, 128:256], x_t[:, t, 128:256], transpose=True
                    )

                    # psm[:, 0:256] = z = x @ Wf^T + b ; psm[:, 256:512] = x_out
                    psm = psm_pool.tile([P, 512], f32)
                    nc.tensor.matmul(
                        psm[:],
                        xt_s[:, 0:128],
                        wc_t[:, 0, :],
                        start=True,
                        stop=False,
                        skip_group_check=True,
                    )
                    nc.tensor.matmul(
                        psm[:, 0:256],
                        ones_t[:],
                        bias_t[:],
                        start=False,
                        stop=False,
                        skip_group_check=True,
                    )
                    nc.tensor.matmul(
                        psm[:],
                        xt_s[:, 128:256],
                        wc_t[:, 1, :],
                        start=False,
                        stop=True,
                        skip_group_check=True,
                    )

                    th = th_pool.tile([P, 256], f32)
                    nc.scalar.activation(
                        th[:], psm[:, 0:256], mybir.ActivationFunctionType.Tanh
                    )
                    nc.vector.tensor_add(res[:, t, :], th[:], psm[:, 256:512])

                nc.sync.dma_start(out_view, res[:])

    # bacc legalization: moves matmul waits to ldweights, splits multi-waits
    # onto event semaphores (HW allows at most 1 wait per instruction).
    nc.finalize()

    _CACHED_NC = nc
    return nc


# ---------------------------------------------------------------------------
# Entry point
# ---------------------------------------------------------------------------

def _prepare_inputs(x, q_phases, proj_w, proj_b):
    x = np.asarray(x, dtype=np.float32)
    q_phases = np.asarray(q_phases)
    proj_w = np.asarray(proj_w, dtype=np.float32)
    proj_b = np.asarray(proj_b, dtype=np.float32)

    import ml_dtypes

    R = _rotation_matrix(q_phases)
    w_fused = proj_w.astype(np.float64) @ R           # [256, 256]
    wc = np.concatenate([w_fused.T, R.T], axis=1)     # [256, 512]
    wc = np.ascontiguousarray(
        wc.reshape(2, P, 512).transpose(1, 0, 2).astype(np.float32)
    ).astype(ml_dtypes.bfloat16)                      # [128, 2, 512]
    bias = proj_b.reshape(1, 256).astype(ml_dtypes.bfloat16)

    in_maps = []
    for c in range(N_CORES):
        shard = np.ascontiguousarray(
            x[c * ROWS_PER_CORE : (c + 1) * ROWS_PER_CORE]
        )
        in_maps.append({"x": shard, "wc": wc, "bias": bias})
    return in_maps, q_phases


def _run(x, q_phases, proj_w, proj_b, trace=False, trace_kwargs=None):
    from concourse.bass_utils import run_bass_kernel_spmd

    in_maps, q_phases = _prepare_inputs(x, q_phases, proj_w, proj_b)
    nc = _build_nc()
    result = run_bass_kernel_spmd(
        nc,
        in_maps,
        core_ids=list(range(N_CORES)),
        trace=trace,
        **(trace_kwargs or {}),
    )
    x_final = np.concatenate([r["out"] for r in result.results], axis=0)
    eta = _compute_eta(q_phases)
    return (x_final, eta), result


def kernel(x, q_phases, proj_w, proj_b):
    (x_final, eta), _ = _run(x, q_phases, proj_w, proj_b, trace=False)
    return x_final, eta


# revision 18
# speedup vs baseline: 4.6969x; 4.6969x over previous
"""Trainium2 Bass kernel for nn_AutonomousSystem_89644557402406.

Computation (see reference):
    S    = su2_operators(q_phases)                 # [64, 2, 2] complex
    eta  = mean(angle(det S)) / pi                 # scalar (det==1 analytically)
    x_out = per-quaternion complex 2x2 rotation of x
          = x @ R^T  with R = blockdiag of 64 real 4x4 blocks
    out  = tanh(x_out @ proj_w.T + proj_b) + x_out

Key restructuring: fold the rotation into the projection on the host:
    W_fused = proj_w @ R   =>   out = tanh(x @ W_fused^T + b) + x @ R^T
so the device does a single [256 -> 512] matmul per row block
(columns 0:256 = pre-tanh z, columns 256:512 = x_out), a tanh and an add.

Sharding: pure data parallel over batch across 8 cores (65536 rows/core).
"""

import math

import numpy as np

MANIFOLD_DIM = 256
NQ = 64
BITS = 4
LEVELS = 2 ** BITS
SCALE = 2.0 * math.pi / (LEVELS - 1)
BATCH = 524288

N_CORES = 8
ROWS_PER_CORE = BATCH // N_CORES          # 65536
P = 128                                    # partitions
SUBTILES = 16                              # 128-row subtiles per outer tile
OUTER_ROWS = P * SUBTILES                  # 2048 rows per outer tile
N_OUTER = ROWS_PER_CORE // OUTER_ROWS      # 32


# ---------------------------------------------------------------------------
# Host-side math: rotation matrix R, fused weights, eta
# ---------------------------------------------------------------------------

def _su2_numpy(q_phases, dtype=np.float64):
    ph = q_phases.astype(dtype) * SCALE
    alpha, beta, gamma = ph[:, 0], ph[:, 1], ph[:, 2]
    cb = np.cos(beta / 2)
    sb = np.sin(beta / 2)
    u00 = np.exp(1j * (alpha + gamma) / 2) * cb
    u01 = np.exp(1j * (alpha - gamma) / 2) * sb
    u10 = -np.exp(-1j * (alpha - gamma) / 2) * sb
    u11 = np.exp(-1j * (alpha + gamma) / 2) * cb
    return np.stack(
        [np.stack([u00, u01], -1), np.stack([u10, u11], -1)], -2
    )  # [NQ, 2, 2]


def _rotation_matrix(q_phases):
    """R such that x_out = x @ R.T (R is block-diagonal, 64 blocks of 4x4)."""
    S = _su2_numpy(q_phases)
    Sr, Si = S.real, S.imag
    blk = np.zeros((NQ, 4, 4), dtype=np.float64)
    # in-block ordering: index 2*j + d with j = complex component, d = re/im
    blk[:, 0::2, 0::2] = Sr
    blk[:, 0::2, 1::2] = -Si
    blk[:, 1::2, 0::2] = Si
    blk[:, 1::2, 1::2] = Sr
    R = np.zeros((MANIFOLD_DIM, MANIFOLD_DIM), dtype=np.float64)
    for q in range(NQ):
        R[4 * q : 4 * q + 4, 4 * q : 4 * q + 4] = blk[q]
    return R


def _compute_eta(q_phases):
    """Replicate the reference eta computation as closely as possible."""
    try:
        import jax
        import jax.numpy as jnp

        cpu = jax.local_devices(backend="cpu")[0]
        with jax.default_device(cpu):
            phases = jnp.asarray(q_phases).astype(jnp.float32) * SCALE
            alpha, beta, gamma = phases[:, 0], phases[:, 1], phases[:, 2]
            cb = jnp.cos(beta / 2)
            sb = jnp.sin(beta / 2)
            u00 = jnp.exp(1j * (alpha + gamma) / 2) * cb
            u01 = jnp.exp(1j * (alpha - gamma) / 2) * sb
            u10 = -jnp.exp(-1j * (alpha - gamma) / 2) * sb
            u11 = jnp.exp(-1j * (alpha + gamma) / 2) * cb
            det = u00 * u11 - u01 * u10
            eta = jnp.angle(det).mean() / math.pi
            return np.asarray(eta, dtype=np.float32)
    except Exception:
        S = _su2_numpy(q_phases, dtype=np.float32).astype(np.complex64)
        det = S[:, 0, 0] * S[:, 1, 1] - S[:, 0, 1] * S[:, 1, 0]
        return np.float32(np.angle(det).mean() / np.float32(math.pi))


# ---------------------------------------------------------------------------
# Bass kernel (per-core program; same NEFF on all 8 cores)
# ---------------------------------------------------------------------------

_CACHED_NC = None


def _build_nc():
    global _CACHED_NC
    if _CACHED_NC is not None:
        return _CACHED_NC

    import concourse.mybir as mybir
    from concourse import bacc
    from concourse.masks import make_identity
    from concourse.tile import TileContext

    f32 = mybir.dt.float32
    bf16 = mybir.dt.bfloat16

    nc = bacc.Bacc("TRN2", target_bir_lowering=False, debug=False)

    # x is f32 in HBM; the input DMA casts to bf16 (SWDGE cast-on-copy).
    # Matmul inputs are bf16 (1 cycle/row on the PE); accumulation is f32.
    x = nc.dram_tensor("x", [ROWS_PER_CORE, MANIFOLD_DIM], f32, kind="ExternalInput")
    # pre-arranged combined weights: wc[p, c, n] = Wc[c*128 + p, n]
    wc = nc.dram_tensor("wc", [P, 2, 512], bf16, kind="ExternalInput")
    bias = nc.dram_tensor("bias", [1, 256], bf16, kind="ExternalInput")
    out = nc.dram_tensor("out", [ROWS_PER_CORE, MANIFOLD_DIM], f32, kind="ExternalOutput")

    with TileContext(nc) as tc:
        with (
            tc.tile_pool(name="const", bufs=1) as cpool,
            tc.tile_pool(name="xin", bufs=3) as xin_pool,
            tc.tile_pool(name="xt", bufs=2) as xt_pool,
            tc.tile_pool(name="res", bufs=3) as res_pool,
            tc.tile_pool(name="psx_p", bufs=4, space="PSUM") as psx_pool,
            tc.tile_pool(name="psm_p", bufs=4, space="PSUM") as psm_pool,
        ):
            wc_t = cpool.tile([P, 2, 512], bf16)
            nc.sync.dma_start(wc_t[:], wc[:, :, :])
            bias_t = cpool.tile([1, 256], bf16)
            nc.sync.dma_start(bias_t[:], bias[:, :])
            ones_t = cpool.tile([1, P], bf16)
            nc.gpsimd.memset(ones_t[:], 1.0)
            ident = cpool.tile([P, P], f32)
            make_identity(nc, ident[:])

            for o in range(N_OUTER):
                row0 = o * OUTER_ROWS
                x_view = x[row0 : row0 + OUTER_ROWS, :].rearrange(
                    "(t p) f -> p t f", p=P
                )
                out_view = out[row0 : row0 + OUTER_ROWS, :].rearrange(
                    "(t p) f -> p t f", p=P
                )

                x_t = xin_pool.tile([P, SUBTILES, MANIFOLD_DIM], bf16)
                # SWDGE dma casts f32 -> bf16 on the fly
                nc.gpsimd.dma_start(out=x_t[:], in_=x_view)

                # Transpose phase: each 128x256 bf16 subtile viewed as
                # 128x128 f32 (bf16 pairs) goes through one PE transpose;
                # 4 subtiles share one PSUM bank, evicted by one fat copy.
                # Resulting layout: xt_b[j, t, 2p+d] = x_t[p, t, 2j+d].
                xt_b = xt_pool.tile([P, SUBTILES, MANIFOLD_DIM], bf16)
                for h in range(SUBTILES // 4):
                    psx = psx_pool.tile([P, 4, P], f32)
                    for j in range(4):
                        t = 4 * h + j
                        nc.tensor.transpose(
                            psx[:, j, :], x_t[:, t, :].bitcast(f32), ident[:]
                        )
                    nc.vector.tensor_copy(
                        xt_b[:, 4 * h : 4 * h + 4, :].bitcast(f32), psx[:]
                    )

                res = res_pool.tile([P, SUBTILES, MANIFOLD_DIM], f32)

                # Matmul phase: dense back-to-back matmuls (keeps PE warm).
                for t in range(SUBTILES):
                    # psm[:, 0:256] = z = x @ Wf^T + b ; psm[:, 256:512] = x_out
                    psm = psm_pool.tile([P, 512], f32)
                    nc.tensor.matmul(
                        psm[:],
                        xt_b[:, t, 0::2],
                        wc_t[:, 0, :],
                        start=True,
                        stop=False,
                        skip_group_check=True,
                    )
                    nc.tensor.matmul(
                        psm[:, 0:256],
                        ones_t[:],
                        bias_t[:],
                        start=False,
                        stop=False,
                        skip_group_check=True,
                    )
                    nc.tensor.matmul(
                        psm[:],
                        xt_b[:, t, 1::2],
                        wc_t[:, 1, :],
                        start=False,
                        stop=True,
                        skip_group_check=True,
                    )

                    nc.scalar.activation(
                        res[:, t, :], psm[:, 0:256],
                        mybir.ActivationFunctionType.Tanh,
                    )
                    nc.vector.tensor_add(
                        res[:, t, :], res[:, t, :], psm[:, 256:512]
                    )

                nc.sync.dma_start(out_view, res[:])

    # bacc legalization: moves matmul waits to ldweights, splits multi-waits
    # onto event semaphores (HW allows at most 1 wait per instruction).
    nc.finalize()

    _CACHED_NC = nc
    return nc


# ---------------------------------------------------------------------------
# Entry point
# ---------------------------------------------------------------------------

def _prepare_inputs(x, q_phases, proj_w, proj_b):
    x = np.asarray(x, dtype=np.float32)
    q_phases = np.asarray(q_phases)
    proj_w = np.asarray(proj_w, dtype=np.float32)
    proj_b = np.asarray(proj_b, dtype=np.float32)

    import ml_dtypes

    R = _rotation_matrix(q_phases)
    w_fused = proj_w.astype(np.float64) @ R           # [256, 256]
    wc = np.concatenate([w_fused.T, R.T], axis=1)     # [256, 512]
    # pair-packed transpose puts features (2p, 2p+1) on partition p, so the
    # even k-chunk is rows {2p} and the odd k-chunk rows {2p+1}:
    # wc[p, d, n] = Wc[2p + d, n]
    wc = np.ascontiguousarray(
        wc.reshape(P, 2, 512).astype(np.float32)
    ).astype(ml_dtypes.bfloat16)                      # [128, 2, 512]
    bias = proj_b.reshape(1, 256).astype(ml_dtypes.bfloat16)

    in_maps = []
    for c in range(N_CORES):
        shard = np.ascontiguousarray(
            x[c * ROWS_PER_CORE : (c + 1) * ROWS_PER_CORE]
        )
        in_maps.append({"x": shard, "wc": wc, "bias": bias})
    return in_maps, q_phases


def _run(x, q_phases, proj_w, proj_b, trace=False, trace_kwargs=None):
    from concourse.bass_utils import run_bass_kernel_spmd

    in_maps, q_phases = _prepare_inputs(x, q_phases, proj_w, proj_b)
    nc = _build_nc()
    result = run_bass_kernel_spmd(
        nc,
        in_maps,
        core_ids=list(range(N_CORES)),
        trace=trace,
        **(trace_kwargs or {}),
    )
    x_final = np.concatenate([r["out"] for r in result.results], axis=0)
    eta = _compute_eta(q_phases)
    return (x_final, eta), result


def kernel(x, q_phases, proj_w, proj_b):
    (x_final, eta), _ = _run(x, q_phases, proj_w, proj_b, trace=False)
    return x_final, eta


# revision 21
# speedup vs baseline: 5.0961x; 1.0850x over previous
"""Trainium2 Bass kernel for nn_AutonomousSystem_89644557402406.

Computation (see reference):
    S    = su2_operators(q_phases)                 # [64, 2, 2] complex
    eta  = mean(angle(det S)) / pi                 # scalar (det==1 analytically)
    x_out = per-quaternion complex 2x2 rotation of x
          = x @ R^T  with R = blockdiag of 64 real 4x4 blocks
    out  = tanh(x_out @ proj_w.T + proj_b) + x_out

Key restructuring: fold the rotation into the projection on the host:
    W_fused = proj_w @ R   =>   out = tanh(x @ W_fused^T + b) + x @ R^T
so the device does a single [256 -> 512] matmul per row block
(columns 0:256 = pre-tanh z, columns 256:512 = x_out), a tanh and an add.

Sharding: pure data parallel over batch across 8 cores (65536 rows/core).
"""

import math

import numpy as np

MANIFOLD_DIM = 256
NQ = 64
BITS = 4
LEVELS = 2 ** BITS
SCALE = 2.0 * math.pi / (LEVELS - 1)
BATCH = 524288

N_CORES = 8
ROWS_PER_CORE = BATCH // N_CORES          # 65536
P = 128                                    # partitions
SUBTILES = 16                              # 128-row subtiles per outer tile
OUTER_ROWS = P * SUBTILES                  # 2048 rows per outer tile
N_OUTER = ROWS_PER_CORE // OUTER_ROWS      # 32


# ---------------------------------------------------------------------------
# Host-side math: rotation matrix R, fused weights, eta
# ---------------------------------------------------------------------------

def _su2_numpy(q_phases, dtype=np.float64):
    ph = q_phases.astype(dtype) * SCALE
    alpha, beta, gamma = ph[:, 0], ph[:, 1], ph[:, 2]
    cb = np.cos(beta / 2)
    sb = np.sin(beta / 2)
    u00 = np.exp(1j * (alpha + gamma) / 2) * cb
    u01 = np.exp(1j * (alpha - gamma) / 2) * sb
    u10 = -np.exp(-1j * (alpha - gamma) / 2) * sb
    u11 = np.exp(-1j * (alpha + gamma) / 2) * cb
    return np.stack(
        [np.stack([u00, u01], -1), np.stack([u10, u11], -1)], -2
    )  # [NQ, 2, 2]


def _rotation_matrix(q_phases):
    """R such that x_out = x @ R.T (R is block-diagonal, 64 blocks of 4x4)."""
    S = _su2_numpy(q_phases)
    Sr, Si = S.real, S.imag
    blk = np.zeros((NQ, 4, 4), dtype=np.float64)
    # in-block ordering: index 2*j + d with j = complex component, d = re/im
    blk[:, 0::2, 0::2] = Sr
    blk[:, 0::2, 1::2] = -Si
    blk[:, 1::2, 0::2] = Si
    blk[:, 1::2, 1::2] = Sr
    R = np.zeros((MANIFOLD_DIM, MANIFOLD_DIM), dtype=np.float64)
    for q in range(NQ):
        R[4 * q : 4 * q + 4, 4 * q : 4 * q + 4] = blk[q]
    return R


def _compute_eta(q_phases):
    """Replicate the reference eta computation as closely as possible."""
    try:
        import jax
        import jax.numpy as jnp

        cpu = jax.local_devices(backend="cpu")[0]
        with jax.default_device(cpu):
            phases = jnp.asarray(q_phases).astype(jnp.float32) * SCALE
            alpha, beta, gamma = phases[:, 0], phases[:, 1], phases[:, 2]
            cb = jnp.cos(beta / 2)
            sb = jnp.sin(beta / 2)
            u00 = jnp.exp(1j * (alpha + gamma) / 2) * cb
            u01 = jnp.exp(1j * (alpha - gamma) / 2) * sb
            u10 = -jnp.exp(-1j * (alpha - gamma) / 2) * sb
            u11 = jnp.exp(-1j * (alpha + gamma) / 2) * cb
            det = u00 * u11 - u01 * u10
            eta = jnp.angle(det).mean() / math.pi
            return np.asarray(eta, dtype=np.float32)
    except Exception:
        S = _su2_numpy(q_phases, dtype=np.float32).astype(np.complex64)
        det = S[:, 0, 0] * S[:, 1, 1] - S[:, 0, 1] * S[:, 1, 0]
        return np.float32(np.angle(det).mean() / np.float32(math.pi))


# ---------------------------------------------------------------------------
# Bass kernel (per-core program; same NEFF on all 8 cores)
# ---------------------------------------------------------------------------

_CACHED_NC = None


def _build_nc():
    global _CACHED_NC
    if _CACHED_NC is not None:
        return _CACHED_NC

    import concourse.mybir as mybir
    from concourse import bacc
    from concourse.masks import make_identity
    from concourse.tile import TileContext

    f32 = mybir.dt.float32
    bf16 = mybir.dt.bfloat16

    nc = bacc.Bacc("TRN2", target_bir_lowering=False, debug=False)

    # x is f32 in HBM; the input DMA casts to bf16 (SWDGE cast-on-copy).
    # Matmul inputs are bf16 (1 cycle/row on the PE); accumulation is f32.
    x = nc.dram_tensor("x", [ROWS_PER_CORE, MANIFOLD_DIM], f32, kind="ExternalInput")
    # pre-arranged combined weights: wc[p, c, n] = Wc[2p + c, n]
    wc = nc.dram_tensor("wc", [P, 2, 512], bf16, kind="ExternalInput")
    biasf = nc.dram_tensor("biasf", [P, 2, 256], f32, kind="ExternalInput")
    out = nc.dram_tensor("out", [ROWS_PER_CORE, MANIFOLD_DIM], f32, kind="ExternalOutput")

    with TileContext(nc) as tc:
        with (
            tc.tile_pool(name="const", bufs=1) as cpool,
            tc.tile_pool(name="xin", bufs=3) as xin_pool,
            tc.tile_pool(name="xt", bufs=2) as xt_pool,
            tc.tile_pool(name="t2p", bufs=4) as t2_pool,
            tc.tile_pool(name="res", bufs=3) as res_pool,
            tc.tile_pool(name="psx_p", bufs=2, space="PSUM") as psx_pool,
            tc.tile_pool(name="psm_p", bufs=3, space="PSUM") as psm_pool,
        ):
            wc_t = cpool.tile([P, 2, 512], bf16)
            nc.sync.dma_start(wc_t[:], wc[:, :, :])
            biasf_t = cpool.tile([P, 2, 256], f32)
            nc.sync.dma_start(biasf_t[:], biasf[:, :, :])
            ident = cpool.tile([P, P], f32)
            make_identity(nc, ident[:])

            for o in range(N_OUTER):
                row0 = o * OUTER_ROWS
                x_view = x[row0 : row0 + OUTER_ROWS, :].rearrange(
                    "(t p) f -> p t f", p=P
                )
                out_view = out[row0 : row0 + OUTER_ROWS, :].rearrange(
                    "(t p) f -> p t f", p=P
                )

                x_t = xin_pool.tile([P, SUBTILES, MANIFOLD_DIM], bf16)
                # SWDGE dma casts f32 -> bf16 on the fly
                nc.gpsimd.dma_start(out=x_t[:], in_=x_view)

                xt_b = xt_pool.tile([P, SUBTILES, MANIFOLD_DIM], bf16)
                res = res_pool.tile([P, SUBTILES, MANIFOLD_DIM], f32)

                # Process 4 subtiles per group: transposes (pair-packed f32
                # through the PE), one fat PSUM->SBUF eviction (ACT), clean
                # 2-stationary matmul runs, batched DVE bias/residual adds.
                for g in range(SUBTILES // 4):
                    psx = psx_pool.tile([P, 4, P], f32)
                    for j in range(4):
                        t = 4 * g + j
                        nc.tensor.transpose(
                            psx[:, j, :], x_t[:, t, :].bitcast(f32), ident[:]
                        )
                    # xt_b[j, t, 2p+d] = x_t[p, t, 2j+d]
                    nc.scalar.copy(
                        xt_b[:, 4 * g : 4 * g + 4, :].bitcast(f32), psx[:]
                    )

                    for u in range(2):
                        t0 = 4 * g + 2 * u
                        psm = psm_pool.tile([P, 2, 512], f32)
                        for v in range(2):
                            t = t0 + v
                            nc.tensor.matmul(
                                psm[:, v, :],
                                xt_b[:, t, 0::2],
                                wc_t[:, 0, :],
                                start=True,
                                stop=False,
                                skip_group_check=True,
                            )
                            nc.tensor.matmul(
                                psm[:, v, :],
                                xt_b[:, t, 1::2],
                                wc_t[:, 1, :],
                                start=False,
                                stop=True,
                                skip_group_check=True,
                            )
                        # z + bias (DVE), tanh (ACT), + x_out (DVE, in place)
                        t2 = t2_pool.tile([P, 2, 256], f32)
                        nc.vector.tensor_add(
                            t2[:], psm[:, :, 0:256], biasf_t[:]
                        )
                        nc.scalar.activation(
                            res[:, t0 : t0 + 2, :], t2[:],
                            mybir.ActivationFunctionType.Tanh,
                        )
                        nc.vector.tensor_add(
                            res[:, t0 : t0 + 2, :],
                            res[:, t0 : t0 + 2, :],
                            psm[:, :, 256:512],
                        )

                nc.sync.dma_start(out_view, res[:])

    # bacc legalization: moves matmul waits to ldweights, splits multi-waits
    # onto event semaphores (HW allows at most 1 wait per instruction).
    nc.finalize()

    _CACHED_NC = nc
    return nc


# ---------------------------------------------------------------------------
# Entry point
# ---------------------------------------------------------------------------

def _prepare_inputs(x, q_phases, proj_w, proj_b):
    x = np.asarray(x, dtype=np.float32)
    q_phases = np.asarray(q_phases)
    proj_w = np.asarray(proj_w, dtype=np.float32)
    proj_b = np.asarray(proj_b, dtype=np.float32)

    import ml_dtypes

    R = _rotation_matrix(q_phases)
    w_fused = proj_w.astype(np.float64) @ R           # [256, 256]
    wc = np.concatenate([w_fused.T, R.T], axis=1)     # [256, 512]
    # pair-packed transpose puts features (2p, 2p+1) on partition p, so the
    # even k-chunk is rows {2p} and the odd k-chunk rows {2p+1}:
    # wc[p, d, n] = Wc[2p + d, n]
    wc = np.ascontiguousarray(
        wc.reshape(P, 2, 512).astype(np.float32)
    ).astype(ml_dtypes.bfloat16)                      # [128, 2, 512]
    # bias replicated across partitions and subtile-pair dim for the DVE add
    biasf = np.ascontiguousarray(
        np.broadcast_to(proj_b.astype(np.float32), (P, 2, 256))
    )

    in_maps = []
    for c in range(N_CORES):
        shard = np.ascontiguousarray(
            x[c * ROWS_PER_CORE : (c + 1) * ROWS_PER_CORE]
        )
        in_maps.append({"x": shard, "wc": wc, "biasf": biasf})
    return in_maps, q_phases


def _run(x, q_phases, proj_w, proj_b, trace=False, trace_kwargs=None):
    from concourse.bass_utils import run_bass_kernel_spmd

    in_maps, q_phases = _prepare_inputs(x, q_phases, proj_w, proj_b)
    nc = _build_nc()
    result = run_bass_kernel_spmd(
        nc,
        in_maps,
        core_ids=list(range(N_CORES)),
        trace=trace,
        **(trace_kwargs or {}),
    )
    x_final = np.concatenate([r["out"] for r in result.results], axis=0)
    eta = _compute_eta(q_phases)
    return (x_final, eta), result


def kernel(x, q_phases, proj_w, proj_b):
    (x_final, eta), _ = _run(x, q_phases, proj_w, proj_b, trace=False)
    return x_final, eta
